# revision 44
# baseline (speedup 1.0000x reference)
"""Trainium2 Bass kernel for a causal self-attention block (GQA + gated value
embedding + RoPE + QK-RMSNorm), sharded over 8 NeuronCores.

Sharding: 8 cores = 2 (batch) x 4 (kv-head groups).  Each core computes, for
its batch b and head-group g (4 q-heads + 1 kv-head):
    q/k/v projections, gated ve addition, RoPE, RMSNorm, causal attention,
    and the partial output projection  y_g @ Wproj[g*512:(g+1)*512, :].
The host sums the 4 per-group partials for each batch (the Wproj
contraction distributes over head groups).

v2: bf16 operand datapath (FWL weight loads), host-side x transpose (no
on-device x transposes), 2-block-lagged q/k transposes (keeps the PE
streaming, HAM stays warm), DVE triangular-mask multiplies instead of
GpSimd affine_select, RMS square-sums and qhat scaling on the Scalar
engine, bf16 partial outputs.

Self-contained: hardcodes shapes; accepts FULL inputs, returns FULL output.
"""

from contextlib import ExitStack

import ml_dtypes
import numpy as np

import concourse.bacc as bacc
import concourse.bass as bass
import concourse.mybir as mybir
import concourse.tile as tile
from concourse.bass_utils import run_bass_kernel_spmd
from concourse.masks import make_identity

F32 = mybir.dt.float32
BF16 = mybir.dt.bfloat16
I32 = mybir.dt.int32
AF = mybir.ActivationFunctionType
ALU = mybir.AluOpType
AX = mybir.AxisListType

B, C, HD, NHL, GC = 2, 2048, 128, 4, 32  # NHL = local q heads per core
EPS = float(np.finfo(np.float32).eps)
ISQ = 1.0 / float(np.sqrt(128.0))
RSQRT_MAGIC = 0x5F3759DF


def _bcast(ap_, idx, count):
    """Insert a step-0 (broadcast) dim at position idx of the AP dims."""
    lst = [list(p) for p in ap_.ap]
    lst.insert(idx, [0, count])
    return bass.AP(ap_.tensor, ap_.offset, lst)


def build(T=2048):
    TB = T // 128   # token blocks
    CT = C // 128   # contraction tiles for qkv
    NCH = T // 512  # i-chunks for attention
    OC = C // 512   # output chunks for proj

    nc = bacc.Bacc("TRN2", target_bir_lowering=False, debug=False)
    xt = nc.dram_tensor("xt", [C, T], BF16, kind="ExternalInput")
    xg = nc.dram_tensor("xg", [T, GC], F32, kind="ExternalInput")
    wqkv = nc.dram_tensor("wqkv", [C, NHL * HD + 2 * HD], BF16, kind="ExternalInput")
    wproj = nc.dram_tensor("wproj", [NHL * HD, C], BF16, kind="ExternalInput")
    wgate = nc.dram_tensor("wgate", [GC, 1], F32, kind="ExternalInput")
    ve2 = nc.dram_tensor("ve2", [T, HD], F32, kind="ExternalInput")
    cosn = nc.dram_tensor("cosn", [T, 64], F32, kind="ExternalInput")
    sinn = nc.dram_tensor("sinn", [T, 64], F32, kind="ExternalInput")
    out = nc.dram_tensor("out", [T, C], BF16, kind="ExternalOutput")
    rscr = nc.dram_tensor("rscr", [NHL, T], F32)  # denominator bounce buffer
    rp2 = nc.dram_tensor("rp2", [NHL, T], F32)    # reciprocal bounce buffer

    QN = NHL * HD          # 512 q cols
    KVN = 2 * HD           # 256 k|v cols

    with ExitStack() as stk:
        tc = stk.enter_context(tile.TileContext(nc))
        gpool = stk.enter_context(tc.tile_pool(name="gconst", bufs=1))
        ident = gpool.tile([128, 128], F32)
        make_identity(nc, ident)
        identb = gpool.tile([128, 128], BF16)
        nc.vector.tensor_copy(out=identb, in_=ident)
        # full-width ones stationary: a [128,1] stationary (col_grp q0)
        # breaks LDWEIGHTS/matmul pipelining, costing ~190ns per jb tile
        ones_f = gpool.tile([128, 128], F32)
        nc.vector.memset(ones_f, 1.0)
        ones_b = gpool.tile([128, 128], BF16)
        nc.vector.tensor_copy(out=ones_b, in_=ones_f)
        # upper-triangular (keep j<=i) causal mask for diagonal 128x128 tiles
        triu_f = gpool.tile([128, 128], F32)
        nc.vector.memset(triu_f, 1.0)
        nc.gpsimd.affine_select(
            out=triu_f, in_=triu_f, pattern=[[1, 128]], compare_op=ALU.is_ge,
            fill=0.0, base=0, channel_multiplier=-1)
        triu_b = gpool.tile([128, 128], BF16)
        nc.vector.tensor_copy(out=triu_b, in_=triu_f)

        # PE warmup: dummy transposes so HAM reaches full clock while the
        # first DMAs land. Uses a memset scratch tile so the first transpose
        # depends only on one dependency-free DVE memset.
        wz = gpool.tile([128, 128], BF16)
        nc.vector.memset(wz, 0.0)
        with tc.tile_pool(name="warm", bufs=2, space="PSUM") as warm:
            for _ in range(112):
                w_ps = warm.tile([128, 128], BF16, tag="wps", name="wps")
                nc.tensor.transpose(w_ps, wz, wz)

        persist = stk.enter_context(tc.tile_pool(name="persist", bufs=1))
        qT = persist.tile([128, NHL, T], BF16)   # [d, h, t]
        kT = persist.tile([128, T], BF16)        # [d, t]
        vS = persist.tile([128, TB, HD], BF16)   # [t%128, t//128, d]
        yT = persist.tile([128, NHL, T], BF16)   # [d, h, t]

        # qkh and pst stay open into phase B: the last two blocks' q/k
        # transposes are emitted inside phase B (they are only needed by the
        # c=3 chunks), so phase B's first S matmuls never wait on phase A's
        # DVE tail.
        qkh = stk.enter_context(tc.tile_pool(name="qkh", bufs=4))
        pst = stk.enter_context(tc.tile_pool(name="pst", bufs=2, space="PSUM"))

        pend = {}  # tb -> (qhat, khat) awaiting transpose into qT/kT

        def drain_pend(tb):
            pqh, pkh = pend.pop(tb)
            t0 = tb * 128
            for hh in range(NHL):
                tq_ps = pst.tile([128, 128], BF16, tag="tps")
                nc.tensor.transpose(
                    tq_ps, pqh[:, hh * 128:(hh + 1) * 128], identb)
                if hh % 2 == 0:
                    nc.scalar.copy(out=qT[:, hh, t0:t0 + 128], in_=tq_ps)
                else:
                    nc.vector.tensor_copy(out=qT[:, hh, t0:t0 + 128], in_=tq_ps)
            tk_ps = pst.tile([128, 128], BF16, tag="tps")
            nc.tensor.transpose(tk_ps, pkh, identb)
            nc.vector.tensor_copy(out=kT[:, t0:t0 + 128], in_=tk_ps)

        # ---------------- phase A: qkv + rope + rmsnorm + transposes --------
        with nc.named_scope("phaseA"), \
                tc.tile_pool(name="wA", bufs=1) as wA, \
                tc.tile_pool(name="xpA", bufs=3) as xpA, \
                tc.tile_pool(name="sbA", bufs=3) as sbA, \
                tc.tile_pool(name="psq", bufs=3, space="PSUM") as psq, \
                tc.tile_pool(name="pskv", bufs=3, space="PSUM") as pskv:
            # x pair 0 first (gates the first matmuls), then the small
            # tables the block-0 DVE chain needs, then the rest; qkv
            # weights stream per-ct on the gpsimd queue in parallel.
            xtr = xt.rearrange("(ct p) t -> p ct t", p=128)
            xpairs = {}

            def load_xpair(pr):
                xp = xpA.tile([128, CT, 256], BF16, tag="xp", name="xp")
                nc.sync.dma_start(out=xp, in_=xtr[:, :, pr * 256:pr * 256 + 256])
                xpairs[pr] = xp

            load_xpair(0)
            wgb_sb = wA.tile([128, GC], F32)
            nc.sync.dma_start(out=wgb_sb, in_=_bcast(wgate[:, 0], 0, 128))
            cos_sb = wA.tile([128, TB, 64], F32)
            nc.sync.dma_start(out=cos_sb, in_=cosn.rearrange("(tb p) d -> p tb d", p=128))
            sin_sb = wA.tile([128, TB, 64], F32)
            nc.sync.dma_start(out=sin_sb, in_=sinn.rearrange("(tb p) d -> p tb d", p=128))
            xg_sb = wA.tile([128, TB, GC], F32)
            nc.sync.dma_start(out=xg_sb, in_=xg.rearrange("(tb p) g -> p tb g", p=128))
            ve_sb = wA.tile([128, TB, HD], F32)
            nc.sync.dma_start(out=ve_sb, in_=ve2.rearrange("(tb p) d -> p tb d", p=128))
            load_xpair(1)
            wqkv_sb = wA.tile([128, CT, QN + KVN], BF16)
            wqkvr = wqkv.rearrange("(ct p) j -> p ct j", p=128)
            for ct in range(CT):
                nc.gpsimd.dma_start(out=wqkv_sb[:, ct, :], in_=wqkvr[:, ct, :])

            # Per-block chain is software-pipelined across blocks: the rope
            # and square-sum of block tb are emitted with tb's matmuls, but
            # the Newton rsqrt / qhat / khat / gate-finish / vS of block tb
            # are deferred one iteration so no engine FIFO ever waits on a
            # cross-engine result of the same block.
            chain = {}

            def finish_chain(tb):
                ch = chain.pop(tb)
                red, qh, kh = ch["red"], ch["qh"], ch["kh"]
                # m = mean + eps; rsqrt via bit-trick seed + 2 Newton steps
                nc.vector.tensor_scalar(
                    out=red, in0=red, scalar1=1.0 / 128.0, scalar2=EPS,
                    op0=ALU.mult, op1=ALU.add)
                rq = sbA.tile([128, NHL + 1], F32, tag="rq")
                rqi = rq.bitcast(I32)
                nc.vector.tensor_scalar(
                    out=rqi, in0=red.bitcast(I32), scalar1=1, scalar2=None,
                    op0=ALU.logical_shift_right)
                nc.vector.tensor_scalar(
                    out=rqi, in0=rqi, scalar1=-1, scalar2=RSQRT_MAGIC,
                    op0=ALU.mult, op1=ALU.add)
                nt = sbA.tile([128, NHL + 1], F32, tag="nt")
                for _ in range(2):
                    nc.vector.tensor_tensor(out=nt, in0=rq, in1=rq, op=ALU.mult)
                    nc.vector.tensor_tensor(out=nt, in0=nt, in1=red, op=ALU.mult)
                    nc.vector.tensor_scalar(
                        out=nt, in0=nt, scalar1=-0.5, scalar2=1.5,
                        op0=ALU.mult, op1=ALU.add)
                    nc.vector.tensor_tensor(out=rq, in0=rq, in1=nt, op=ALU.mult)
                qhat = qkh.tile([128, NHL * HD], BF16, tag="qhat")
                rqB = _bcast(rq[:, 0:NHL], 2, HD)
                nc.vector.tensor_tensor(
                    out=qhat.rearrange("p (h d) -> p h d", h=NHL),
                    in0=qh.rearrange("p (h d) -> p h d", h=NHL),
                    in1=rqB, op=ALU.mult)
                khat = qkh.tile([128, HD], BF16, tag="khat")
                nc.vector.tensor_scalar_mul(khat, kh, rq[:, NHL:NHL + 1])
                # gate finish + v = v_mm + sigmoid(z) * (2*ve)
                e_sb = ch["e"]
                nc.vector.tensor_scalar_add(e_sb, e_sb, 1.0)
                g_sb = sbA.tile([128, 1], F32, tag="gsb")
                nc.vector.reciprocal(g_sb, e_sb)
                nc.vector.scalar_tensor_tensor(
                    out=vS[:, ch["tb"], :], in0=ve_sb[:, ch["tb"], :],
                    scalar=g_sb, in1=ch["kv"][:, HD:2 * HD],
                    op0=ALU.mult, op1=ALU.add)
                pend[tb] = (qhat, khat)

            for tb in range(TB):
                pr, half = divmod(tb, 2)
                if pr + 1 not in xpairs and pr + 1 < TB // 2:
                    load_xpair(pr + 1)
                xp = xpairs[pr]
                q_ps = psq.tile([128, QN], F32, tag="qps")
                kv_ps = pskv.tile([128, KVN], F32, tag="kvps")
                for ct in range(CT):
                    xl = xp[:, ct, half * 128:half * 128 + 128]
                    nc.tensor.matmul(
                        q_ps, lhsT=xl, rhs=wqkv_sb[:, ct, 0:QN],
                        start=(ct == 0), stop=(ct == CT - 1))
                    nc.tensor.matmul(
                        kv_ps, lhsT=xl, rhs=wqkv_sb[:, ct, QN:QN + KVN],
                        start=(ct == 0), stop=(ct == CT - 1))
                # previous-previous block's qhat/khat -> qT/kT (PE transposes)
                if tb - 2 in pend:
                    drain_pend(tb - 2)
                if half == 1 and pr in xpairs:
                    del xpairs[pr]

                # gate part 1: z = x[:, :32] @ wgate (DVE), e = exp(-z) (Act)
                zg_sb = sbA.tile([128, 1], F32, tag="zg")
                zscr = sbA.tile([128, GC], F32, tag="zscr")
                nc.vector.scalar_tensor_tensor(
                    out=zscr, in0=xg_sb[:, tb, :], scalar=1.0, in1=wgb_sb,
                    op0=ALU.bypass, op1=ALU.mult, accum_out=zg_sb)
                e_sb = sbA.tile([128, 1], F32, tag="esb")
                nc.scalar.activation(e_sb, zg_sb, AF.Exp, scale=-1.0)

                # ---- RoPE on q (4 heads batched) and k ----
                cosB = _bcast(cos_sb[:, tb, :], 1, NHL)
                sinB = _bcast(sin_sb[:, tb, :], 1, NHL)
                qv = q_ps.rearrange("p (h d) -> p h d", h=NHL)
                qh = sbA.tile([128, NHL * HD], F32, tag="qh")
                qhv = qh.rearrange("p (h d) -> p h d", h=NHL)
                tmp = sbA.tile([128, NHL, 64], F32, tag="tmp")
                nc.vector.tensor_tensor(
                    out=qhv[:, :, 0:64], in0=qv[:, :, 0:64], in1=cosB, op=ALU.mult)
                nc.vector.tensor_tensor(
                    out=tmp, in0=qv[:, :, 64:128], in1=sinB, op=ALU.mult)
                nc.vector.tensor_tensor(
                    out=qhv[:, :, 0:64], in0=qhv[:, :, 0:64], in1=tmp, op=ALU.add)
                nc.vector.tensor_tensor(
                    out=qhv[:, :, 64:128], in0=qv[:, :, 64:128], in1=cosB, op=ALU.mult)
                nc.vector.tensor_tensor(
                    out=tmp, in0=qv[:, :, 0:64], in1=sinB, op=ALU.mult)
                nc.vector.tensor_tensor(
                    out=qhv[:, :, 64:128], in0=qhv[:, :, 64:128], in1=tmp,
                    op=ALU.subtract)
                kv = kv_ps[:, 0:HD]
                kh = sbA.tile([128, HD], F32, tag="kh")
                ktmp = sbA.tile([128, 64], F32, tag="ktmp")
                cs1 = cos_sb[:, tb, :]
                sn1 = sin_sb[:, tb, :]
                nc.vector.tensor_tensor(
                    out=kh[:, 0:64], in0=kv[:, 0:64], in1=cs1, op=ALU.mult)
                nc.vector.tensor_tensor(
                    out=ktmp, in0=kv[:, 64:128], in1=sn1, op=ALU.mult)
                nc.vector.tensor_tensor(
                    out=kh[:, 0:64], in0=kh[:, 0:64], in1=ktmp, op=ALU.add)
                nc.vector.tensor_tensor(
                    out=kh[:, 64:128], in0=kv[:, 64:128], in1=cs1, op=ALU.mult)
                nc.vector.tensor_tensor(
                    out=ktmp, in0=kv[:, 0:64], in1=sn1, op=ALU.mult)
                nc.vector.tensor_tensor(
                    out=kh[:, 64:128], in0=kh[:, 64:128], in1=ktmp, op=ALU.subtract)

                # ---- RMSNorm: per-head sum of squares on the Scalar engine
                red = sbA.tile([128, NHL + 1], F32, tag="red")
                sqscr = sbA.tile([128, HD], F32, tag="sqscr")
                for hh in range(NHL):
                    nc.scalar.activation(
                        sqscr, qh[:, hh * 128:(hh + 1) * 128], AF.Square,
                        accum_out=red[:, hh:hh + 1])
                nc.scalar.activation(
                    sqscr, kh, AF.Square, accum_out=red[:, NHL:NHL + 1])

                # deferred finish of the previous block (all inputs one
                # block old -> no DVE FIFO stalls)
                if tb - 1 in chain:
                    finish_chain(tb - 1)
                chain[tb] = dict(tb=tb, red=red, qh=qh, kh=kh, e=e_sb,
                                 kv=kv_ps)

            finish_chain(TB - 1)

        # wproj loaded early so phase C does not stall on it
        wC = stk.enter_context(tc.tile_pool(name="wC", bufs=1))
        wp_sb = wC.tile([128, NHL, OC, 512], BF16)
        nc.gpsimd.dma_start(
            out=wp_sb,
            in_=wproj.rearrange("(h p) (oc o) -> p h oc o", p=128, o=512))

        # ---------------- phase B: attention ----------------
        # Globally software-pipelined: S/exp of group k+1 is emitted before
        # AV/denominator of group k, across iteration boundaries.
        # Denominator handling is deferred to once per head: yps/dps are
        # copied out of PSUM immediately (fast chunk turnover, PE never
        # waits on the reciprocal), then one DRAM round trip reshapes d to
        # partition-major for a cheap [128,16] reciprocal, and the scaled
        # yT write overlaps the next head's matmuls.
        with nc.named_scope("phaseB"), \
                tc.tile_pool(name="ptB", bufs=6) as ptB, \
                tc.tile_pool(name="ysB", bufs=2) as ysB, \
                tc.tile_pool(name="dsB", bufs=2) as dsB, \
                tc.tile_pool(name="bcB", bufs=8) as bcB, \
                tc.tile_pool(name="dpB", bufs=2) as dpB, \
                tc.tile_pool(name="psS", bufs=2, space="PSUM") as psS, \
                tc.tile_pool(name="psy", bufs=1, space="PSUM") as psy, \
                tc.tile_pool(name="psd", bufs=1, space="PSUM") as psd:

            def s_group(meta, g):
                hh, c, i0 = meta["hh"], meta["c"], meta["i0"]
                sps = psS.tile([128, 1024], F32, tag="sps", name="sps")
                pt = ptB.tile([128, 1024], BF16, tag="pt", name="pt")
                for s in range(2):
                    jb = 2 * g + s
                    io = max(0, 128 * jb - 512 * c)  # first causally-live col
                    nc.tensor.matmul(
                        sps[:, s * 512 + io:(s + 1) * 512],
                        lhsT=kT[:, jb * 128:(jb + 1) * 128],
                        rhs=qT[:, hh, i0 + io:i0 + 512],
                        start=True, stop=True)
                # one exp from the first live column: stale columns are never
                # read downstream (AV/dps start at io), so a single big exp
                # is cheaper than per-tile trimmed ones.
                io0 = max(0, 128 * (2 * g - 4 * c))
                nc.scalar.activation(
                    pt[:, io0:1024], sps[:, io0:1024], AF.Exp, scale=ISQ)
                for s in range(2):
                    jb = 2 * g + s
                    if jb >= 4 * c:  # diagonal block: zero j > i (tri mask)
                        io = 128 * (jb - 4 * c)
                        nc.vector.tensor_tensor(
                            out=pt[:, s * 512 + io:s * 512 + io + 128],
                            in0=pt[:, s * 512 + io:s * 512 + io + 128],
                            in1=triu_b, op=ALU.mult)
                meta["pts"][g] = pt

            def av_group(meta, g, is_last):
                pt = meta["pts"].pop(g)
                yps, dps = meta["yps"], meta["dps"]
                c = meta["c"]
                for s in range(2):
                    jb = 2 * g + s
                    io = max(0, 128 * jb - 512 * c)
                    if jb == meta["first_jb"]:
                        io = 0  # start matmul must cover the full chunk
                    nc.tensor.matmul(
                        yps[:, io:512], lhsT=vS[:, jb, :],
                        rhs=pt[:, s * 512 + io:(s + 1) * 512],
                        start=(jb == meta["first_jb"]),
                        stop=(jb == meta["last_jb"]))
                    nc.tensor.matmul(
                        dps[:, io:512], lhsT=ones_b,
                        rhs=pt[:, s * 512 + io:(s + 1) * 512],
                        start=(jb == meta["first_jb"]),
                        stop=(jb == meta["last_jb"]))
                if is_last:
                    hh, c = meta["hh"], meta["c"]
                    nc.vector.tensor_copy(out=meta["ysb"][:, c, :], in_=yps)
                    nc.vector.tensor_copy(
                        out=meta["dsb"][0:1, c, :], in_=dps[0:1, :])
                    if hh == NHL - 1:
                        # last head: staged per-chunk finalize so phase C is
                        # not serialized behind the whole head's round trip
                        s0 = c * 512
                        nc.sync.dma_start(
                            out=rscr[hh, s0:s0 + 512], in_=meta["dsb"][0:1, c, :])
                        dPc = dpB.tile([128, 4], F32, tag="dpc", name="dpc")
                        nc.sync.dma_start(
                            out=dPc,
                            in_=rscr[hh, s0:s0 + 512].rearrange(
                                "(p f) -> p f", p=128))
                        deferred.append(
                            [lambda hh=hh, c=c, dPc=dPc: fin_recip(hh, c, dPc),
                             lambda hh=hh, c=c, ysb=meta["ysb"]:
                                 fin_mult(hh, c, ysb)])
                    elif c == NCH - 1:
                        head_finalize(hh, meta["ysb"], meta["dsb"])

            def fin_recip(hh, c, dPc):
                s0 = c * 512
                rPc = dpB.tile([128, 4], F32, tag="rpc", name="rpc")
                nc.vector.reciprocal(rPc, dPc)
                nc.sync.dma_start(
                    out=rp2[hh, s0:s0 + 512].rearrange("(p f) -> p f", p=128),
                    in_=rPc)
                bca = bcB.tile([128, 512], F32, tag="bca", name="bca")
                nc.sync.dma_start(
                    out=bca, in_=_bcast(rp2[hh, s0:s0 + 512], 0, 128))
                bcas[(hh, c)] = bca

            def fin_mult(hh, c, ysb):
                nc.vector.tensor_tensor(
                    out=yT[:, hh, c * 512:(c + 1) * 512],
                    in0=ysb[:, c, :], in1=bcas.pop((hh, c)), op=ALU.mult)

            def head_finalize(hh, ysb, dsb):
                # d -> DRAM -> partition-major [128,16] -> recip -> DRAM ->
                # per-chunk broadcast; the yT scale multiplies are deferred
                # into the next head's chunk iterations so stalled DVE ops
                # never head-of-line-block the next head's PSUM copies.
                nc.sync.dma_start(out=rscr[hh, :], in_=dsb[0:1, :, :])
                dP = dpB.tile([128, T // 128], F32, tag="dp", name="dp")
                nc.sync.dma_start(
                    out=dP, in_=rscr[hh, :].rearrange("(p f) -> p f", p=128))
                rP = dpB.tile([128, T // 128], F32, tag="rp", name="rp")
                nc.vector.reciprocal(rP, dP)
                nc.sync.dma_start(
                    out=rp2[hh, :].rearrange("(p f) -> p f", p=128), in_=rP)
                for c2 in range(NCH):
                    bca = bcB.tile([128, 512], F32, tag="bca", name="bca")
                    nc.sync.dma_start(
                        out=bca,
                        in_=_bcast(rp2[hh, c2 * 512:(c2 + 1) * 512], 0, 128))
                    bcas[(hh, c2)] = bca
                    deferred.append(
                        [lambda hh=hh, c2=c2, ysb=ysb: fin_mult(hh, c2, ysb)])

            deferred = []  # lists of closures; one stage emitted per chunk
            bcas = {}

            def run_deferred():
                # emit a single deferred stage per chunk boundary: stalled
                # DVE ops must never queue up ahead of the copies that free
                # PSUM for the next chunk
                if deferred:
                    item = deferred[0]
                    item.pop(0)()
                    if not item:
                        deferred.pop(0)

            prev = None
            for hh in range(NHL):
                ysb = ysB.tile([128, NCH, 512], F32, tag="ysb", name="ysb")
                dsb = dsB.tile([1, NCH, 512], F32, tag="dsb", name="dsb")
                for c in range(NCH):
                    if hh == 0 and c == NCH - 1:
                        # blocks 14/15's q/k transposes, deferred from phase
                        # A: needed first by this c=3 chunk, and by now the
                        # phase-A DVE tail has long finished
                        for tbx in sorted(pend):
                            drain_pend(tbx)
                    yps = psy.tile([128, 512], F32, tag="yps", name="yps")
                    dps = psd.tile([128, 512], F32, tag="dps", name="dps")
                    ngrp = (4 * c + 4) // 2
                    order = list(range(ngrp - 2, ngrp)) + list(range(ngrp - 2))
                    meta = dict(hh=hh, c=c, i0=c * 512, yps=yps, dps=dps,
                                ysb=ysb, dsb=dsb,
                                pts={}, first_jb=2 * order[0],
                                last_jb=2 * order[-1] + 1)
                    for idx, g in enumerate(order):
                        s_group(meta, g)
                        if prev is not None:
                            av_group(*prev)
                        run_deferred()
                        prev = (meta, g, idx == len(order) - 1)
            av_group(*prev)
            while deferred:
                run_deferred()

        # ---------------- phase C: output projection ----------------
        with nc.named_scope("phaseC"), \
                tc.tile_pool(name="sbC", bufs=3) as sbC, \
                tc.tile_pool(name="psC", bufs=2, space="PSUM") as psC:
            for tb in range(TB):
                t0 = tb * 128
                for ocp in range(OC // 2):
                    o_ps = psC.tile([128, 1024], F32, tag="ops")
                    for oc2 in range(2):
                        oc = 2 * ocp + oc2
                        for hh in range(NHL):
                            nc.tensor.matmul(
                                o_ps[:, oc2 * 512:(oc2 + 1) * 512],
                                lhsT=yT[:, hh, t0:t0 + 128],
                                rhs=wp_sb[:, hh, oc, :],
                                start=(hh == 0), stop=(hh == NHL - 1))
                    o_sb = sbC.tile([128, 1024], BF16, tag="osb")
                    for oc2 in range(2):
                        oc = 2 * ocp + oc2
                        if oc2 == 0:
                            nc.scalar.copy(
                                out=o_sb[:, oc2 * 512:(oc2 + 1) * 512],
                                in_=o_ps[:, oc2 * 512:(oc2 + 1) * 512])
                        else:
                            nc.vector.tensor_copy(
                                out=o_sb[:, oc2 * 512:(oc2 + 1) * 512],
                                in_=o_ps[:, oc2 * 512:(oc2 + 1) * 512])
                        eng = nc.sync if oc % 2 == 0 else nc.scalar
                        eng.dma_start(
                            out=out[t0:t0 + 128, oc * 512:(oc + 1) * 512],
                            in_=o_sb[:, oc2 * 512:(oc2 + 1) * 512])

    nc.compile()
    return nc


_NC_CACHE = {}


def get_nc(T=2048):
    if T not in _NC_CACHE:
        _NC_CACHE[T] = build(T)
    return _NC_CACHE[T]


def make_in_maps(x, ve, cos, sin, Wq, Wk, Wv, Wproj, Wgate):
    """Shard full inputs into 8 per-core input maps (2 batch x 4 head groups)."""
    BF = ml_dtypes.bfloat16
    x = np.asarray(x, np.float32)
    ve = np.asarray(ve, np.float32)
    cosn = np.ascontiguousarray(np.asarray(cos, np.float32)[0, :, 0, :])
    sinn = np.ascontiguousarray(np.asarray(sin, np.float32)[0, :, 0, :])
    Wq = np.asarray(Wq, np.float32)
    Wk = np.asarray(Wk, np.float32)
    Wv = np.asarray(Wv, np.float32)
    Wproj = np.asarray(Wproj, np.float32)
    Wgate = np.asarray(Wgate, np.float32)
    in_maps = []
    for core in range(8):
        b, g = divmod(core, 4)
        wqkv = np.concatenate(
            [Wq[:, g * 512:(g + 1) * 512],
             Wk[:, g * 128:(g + 1) * 128],
             Wv[:, g * 128:(g + 1) * 128]], axis=1)
        in_maps.append({
            "xt": np.ascontiguousarray(x[b].T).astype(BF),
            "xg": np.ascontiguousarray(x[b][:, :GC]),
            "wqkv": np.ascontiguousarray(wqkv).astype(BF),
            "wproj": np.ascontiguousarray(Wproj[g * 512:(g + 1) * 512, :]).astype(BF),
            "wgate": np.ascontiguousarray(Wgate[:, g:g + 1]),
            "ve2": np.ascontiguousarray(2.0 * ve[b][:, g * 128:(g + 1) * 128]),
            "cosn": cosn,
            "sinn": sinn,
        })
    return in_maps


def run_cores(in_maps, trace=False, **kw):
    nc = get_nc(in_maps[0]["xg"].shape[0])
    return run_bass_kernel_spmd(nc, in_maps, core_ids=list(range(8)), trace=trace, **kw)


def kernel(**inputs):
    in_maps = make_in_maps(
        inputs["x"], inputs["ve"], inputs["cos"], inputs["sin"],
        inputs["Wq"], inputs["Wk"], inputs["Wv"], inputs["Wproj"], inputs["Wgate"])
    res = run_cores(in_maps)
    parts = [np.asarray(res.results[i]["out"], np.float32) for i in range(8)]
    out = np.stack([
        parts[0] + parts[1] + parts[2] + parts[3],
        parts[4] + parts[5] + parts[6] + parts[7],
    ]).astype(np.float32)
    return out


# revision 45
# speedup vs baseline: 1.0205x; 1.0205x over previous
"""Trainium2 Bass kernel for a causal self-attention block (GQA + gated value
embedding + RoPE + QK-RMSNorm), sharded over 8 NeuronCores.

Sharding: 8 cores = 2 (batch) x 4 (kv-head groups).  Each core computes, for
its batch b and head-group g (4 q-heads + 1 kv-head):
    q/k/v projections, gated ve addition, RoPE, RMSNorm, causal attention,
    and the partial output projection  y_g @ Wproj[g*512:(g+1)*512, :].
The host sums the 4 per-group partials for each batch (the Wproj
contraction distributes over head groups).

v2: bf16 operand datapath (FWL weight loads), host-side x transpose (no
on-device x transposes), 2-block-lagged q/k transposes (keeps the PE
streaming, HAM stays warm), DVE triangular-mask multiplies instead of
GpSimd affine_select, RMS square-sums and qhat scaling on the Scalar
engine, bf16 partial outputs.

Self-contained: hardcodes shapes; accepts FULL inputs, returns FULL output.
"""

from contextlib import ExitStack

import ml_dtypes
import numpy as np

import concourse.bacc as bacc
import concourse.bass as bass
import concourse.mybir as mybir
import concourse.tile as tile
from concourse.bass_utils import run_bass_kernel_spmd
from concourse.masks import make_identity

F32 = mybir.dt.float32
BF16 = mybir.dt.bfloat16
I32 = mybir.dt.int32
AF = mybir.ActivationFunctionType
ALU = mybir.AluOpType
AX = mybir.AxisListType

B, C, HD, NHL, GC = 2, 2048, 128, 4, 32  # NHL = local q heads per core
EPS = float(np.finfo(np.float32).eps)
ISQ = 1.0 / float(np.sqrt(128.0))
RSQRT_MAGIC = 0x5F3759DF


def _bcast(ap_, idx, count):
    """Insert a step-0 (broadcast) dim at position idx of the AP dims."""
    lst = [list(p) for p in ap_.ap]
    lst.insert(idx, [0, count])
    return bass.AP(ap_.tensor, ap_.offset, lst)


def build(T=2048):
    TB = T // 128   # token blocks
    CT = C // 128   # contraction tiles for qkv
    NCH = T // 512  # i-chunks for attention
    OC = C // 512   # output chunks for proj

    nc = bacc.Bacc("TRN2", target_bir_lowering=False, debug=False)
    xt = nc.dram_tensor("xt", [C, T], BF16, kind="ExternalInput")
    xg = nc.dram_tensor("xg", [T, GC], F32, kind="ExternalInput")
    wqkv = nc.dram_tensor("wqkv", [C, NHL * HD + 2 * HD], BF16, kind="ExternalInput")
    wproj = nc.dram_tensor("wproj", [NHL * HD, C], BF16, kind="ExternalInput")
    wgate = nc.dram_tensor("wgate", [GC, 1], F32, kind="ExternalInput")
    ve2 = nc.dram_tensor("ve2", [T, HD], F32, kind="ExternalInput")
    cosn = nc.dram_tensor("cosn", [T, 64], F32, kind="ExternalInput")
    sinn = nc.dram_tensor("sinn", [T, 64], F32, kind="ExternalInput")
    out = nc.dram_tensor("out", [T, C], BF16, kind="ExternalOutput")
    rscr = nc.dram_tensor("rscr", [NHL, T], F32)  # denominator bounce buffer
    rp2 = nc.dram_tensor("rp2", [NHL, T], F32)    # reciprocal bounce buffer

    QN = NHL * HD          # 512 q cols
    KVN = 2 * HD           # 256 k|v cols

    with ExitStack() as stk:
        tc = stk.enter_context(tile.TileContext(nc))
        gpool = stk.enter_context(tc.tile_pool(name="gconst", bufs=1))
        ident = gpool.tile([128, 128], F32)
        make_identity(nc, ident)
        identb = gpool.tile([128, 128], BF16)
        nc.vector.tensor_copy(out=identb, in_=ident)
        # full-width ones stationary: a [128,1] stationary (col_grp q0)
        # breaks LDWEIGHTS/matmul pipelining, costing ~190ns per jb tile
        ones_f = gpool.tile([128, 128], F32)
        nc.vector.memset(ones_f, 1.0)
        ones_b = gpool.tile([128, 128], BF16)
        nc.vector.tensor_copy(out=ones_b, in_=ones_f)
        # upper-triangular (keep j<=i) causal mask for diagonal 128x128 tiles
        triu_f = gpool.tile([128, 128], F32)
        nc.vector.memset(triu_f, 1.0)
        nc.gpsimd.affine_select(
            out=triu_f, in_=triu_f, pattern=[[1, 128]], compare_op=ALU.is_ge,
            fill=0.0, base=0, channel_multiplier=-1)
        triu_b = gpool.tile([128, 128], BF16)
        nc.vector.tensor_copy(out=triu_b, in_=triu_f)

        # PE warmup: dummy transposes so HAM reaches full clock while the
        # first DMAs land.
        with tc.tile_pool(name="warm", bufs=2, space="PSUM") as warm:
            for _ in range(112):
                w_ps = warm.tile([128, 128], BF16, tag="wps", name="wps")
                nc.tensor.transpose(w_ps, identb, identb)

        persist = stk.enter_context(tc.tile_pool(name="persist", bufs=1))
        qT = persist.tile([128, NHL, T], BF16)   # [d, h, t]
        kT = persist.tile([128, T], BF16)        # [d, t]
        vS = persist.tile([128, TB, HD], BF16)   # [t%128, t//128, d]
        yT = persist.tile([128, NHL, T], BF16)   # [d, h, t]

        # qkh and pst stay open into phase B: the last two blocks' q/k
        # transposes are emitted inside phase B (they are only needed by the
        # c=3 chunks), so phase B's first S matmuls never wait on phase A's
        # DVE tail.
        qkh = stk.enter_context(tc.tile_pool(name="qkh", bufs=4))
        pst = stk.enter_context(tc.tile_pool(name="pst", bufs=2, space="PSUM"))

        pend = {}  # tb -> (qhat, khat) awaiting transpose into qT/kT

        def drain_pend(tb):
            pqh, pkh = pend.pop(tb)
            t0 = tb * 128
            for hh in range(NHL):
                tq_ps = pst.tile([128, 128], BF16, tag="tps")
                nc.tensor.transpose(
                    tq_ps, pqh[:, hh * 128:(hh + 1) * 128], identb)
                if hh % 2 == 0:
                    nc.scalar.copy(out=qT[:, hh, t0:t0 + 128], in_=tq_ps)
                else:
                    nc.vector.tensor_copy(out=qT[:, hh, t0:t0 + 128], in_=tq_ps)
            tk_ps = pst.tile([128, 128], BF16, tag="tps")
            nc.tensor.transpose(tk_ps, pkh, identb)
            nc.vector.tensor_copy(out=kT[:, t0:t0 + 128], in_=tk_ps)

        # ---------------- phase A: qkv + rope + rmsnorm + transposes --------
        with nc.named_scope("phaseA"), \
                tc.tile_pool(name="wA", bufs=1) as wA, \
                tc.tile_pool(name="xpA", bufs=3) as xpA, \
                tc.tile_pool(name="sbA", bufs=3) as sbA, \
                tc.tile_pool(name="psq", bufs=3, space="PSUM") as psq, \
                tc.tile_pool(name="pskv", bufs=3, space="PSUM") as pskv:
            # x pair 0 first (gates the first matmuls), then the small
            # tables the block-0 DVE chain needs, then the rest; qkv
            # weights stream per-ct on the gpsimd queue in parallel.
            xtr = xt.rearrange("(ct p) t -> p ct t", p=128)
            xpairs = {}

            def load_xpair(pr):
                xp = xpA.tile([128, CT, 256], BF16, tag="xp", name="xp")
                nc.sync.dma_start(out=xp, in_=xtr[:, :, pr * 256:pr * 256 + 256])
                xpairs[pr] = xp

            load_xpair(0)
            wgb_sb = wA.tile([128, GC], F32)
            nc.sync.dma_start(out=wgb_sb, in_=_bcast(wgate[:, 0], 0, 128))
            cos_sb = wA.tile([128, TB, 64], F32)
            nc.sync.dma_start(out=cos_sb, in_=cosn.rearrange("(tb p) d -> p tb d", p=128))
            sin_sb = wA.tile([128, TB, 64], F32)
            nc.sync.dma_start(out=sin_sb, in_=sinn.rearrange("(tb p) d -> p tb d", p=128))
            xg_sb = wA.tile([128, TB, GC], F32)
            nc.sync.dma_start(out=xg_sb, in_=xg.rearrange("(tb p) g -> p tb g", p=128))
            ve_sb = wA.tile([128, TB, HD], F32)
            nc.sync.dma_start(out=ve_sb, in_=ve2.rearrange("(tb p) d -> p tb d", p=128))
            load_xpair(1)
            wqkv_sb = wA.tile([128, CT, QN + KVN], BF16)
            wqkvr = wqkv.rearrange("(ct p) j -> p ct j", p=128)
            for ct in range(CT):
                nc.gpsimd.dma_start(out=wqkv_sb[:, ct, :], in_=wqkvr[:, ct, :])

            # Per-block chain is software-pipelined across blocks: the rope
            # and square-sum of block tb are emitted with tb's matmuls, but
            # the Newton rsqrt / qhat / khat / gate-finish / vS of block tb
            # are deferred one iteration so no engine FIFO ever waits on a
            # cross-engine result of the same block.
            chain = {}

            def finish_chain(tb):
                ch = chain.pop(tb)
                red, qh, kh = ch["red"], ch["qh"], ch["kh"]
                # m = mean + eps; rsqrt via bit-trick seed + 2 Newton steps
                nc.vector.tensor_scalar(
                    out=red, in0=red, scalar1=1.0 / 128.0, scalar2=EPS,
                    op0=ALU.mult, op1=ALU.add)
                rq = sbA.tile([128, NHL + 1], F32, tag="rq")
                rqi = rq.bitcast(I32)
                nc.vector.tensor_scalar(
                    out=rqi, in0=red.bitcast(I32), scalar1=1, scalar2=None,
                    op0=ALU.logical_shift_right)
                nc.vector.tensor_scalar(
                    out=rqi, in0=rqi, scalar1=-1, scalar2=RSQRT_MAGIC,
                    op0=ALU.mult, op1=ALU.add)
                nt = sbA.tile([128, NHL + 1], F32, tag="nt")
                for _ in range(2):
                    nc.vector.tensor_tensor(out=nt, in0=rq, in1=rq, op=ALU.mult)
                    nc.vector.tensor_tensor(out=nt, in0=nt, in1=red, op=ALU.mult)
                    nc.vector.tensor_scalar(
                        out=nt, in0=nt, scalar1=-0.5, scalar2=1.5,
                        op0=ALU.mult, op1=ALU.add)
                    nc.vector.tensor_tensor(out=rq, in0=rq, in1=nt, op=ALU.mult)
                qhat = qkh.tile([128, NHL * HD], BF16, tag="qhat")
                rqB = _bcast(rq[:, 0:NHL], 2, HD)
                nc.vector.tensor_tensor(
                    out=qhat.rearrange("p (h d) -> p h d", h=NHL),
                    in0=qh.rearrange("p (h d) -> p h d", h=NHL),
                    in1=rqB, op=ALU.mult)
                khat = qkh.tile([128, HD], BF16, tag="khat")
                nc.vector.tensor_scalar_mul(khat, kh, rq[:, NHL:NHL + 1])
                # gate finish + v = v_mm + sigmoid(z) * (2*ve)
                e_sb = ch["e"]
                nc.vector.tensor_scalar_add(e_sb, e_sb, 1.0)
                g_sb = sbA.tile([128, 1], F32, tag="gsb")
                nc.vector.reciprocal(g_sb, e_sb)
                nc.vector.scalar_tensor_tensor(
                    out=vS[:, ch["tb"], :], in0=ve_sb[:, ch["tb"], :],
                    scalar=g_sb, in1=ch["kv"][:, HD:2 * HD],
                    op0=ALU.mult, op1=ALU.add)
                pend[tb] = (qhat, khat)

            for tb in range(TB):
                pr, half = divmod(tb, 2)
                if pr + 1 not in xpairs and pr + 1 < TB // 2:
                    load_xpair(pr + 1)
                xp = xpairs[pr]
                q_ps = psq.tile([128, QN], F32, tag="qps")
                kv_ps = pskv.tile([128, KVN], F32, tag="kvps")
                for ct in range(CT):
                    xl = xp[:, ct, half * 128:half * 128 + 128]
                    nc.tensor.matmul(
                        q_ps, lhsT=xl, rhs=wqkv_sb[:, ct, 0:QN],
                        start=(ct == 0), stop=(ct == CT - 1))
                    nc.tensor.matmul(
                        kv_ps, lhsT=xl, rhs=wqkv_sb[:, ct, QN:QN + KVN],
                        start=(ct == 0), stop=(ct == CT - 1))
                # previous-previous block's qhat/khat -> qT/kT (PE transposes)
                if tb - 2 in pend:
                    drain_pend(tb - 2)
                if half == 1 and pr in xpairs:
                    del xpairs[pr]

                # gate part 1: z = x[:, :32] @ wgate (DVE), e = exp(-z) (Act)
                zg_sb = sbA.tile([128, 1], F32, tag="zg")
                zscr = sbA.tile([128, GC], F32, tag="zscr")
                nc.vector.scalar_tensor_tensor(
                    out=zscr, in0=xg_sb[:, tb, :], scalar=1.0, in1=wgb_sb,
                    op0=ALU.bypass, op1=ALU.mult, accum_out=zg_sb)
                e_sb = sbA.tile([128, 1], F32, tag="esb")
                nc.scalar.activation(e_sb, zg_sb, AF.Exp, scale=-1.0)

                # ---- RoPE on q (4 heads batched) and k ----
                cosB = _bcast(cos_sb[:, tb, :], 1, NHL)
                sinB = _bcast(sin_sb[:, tb, :], 1, NHL)
                qv = q_ps.rearrange("p (h d) -> p h d", h=NHL)
                qh = sbA.tile([128, NHL * HD], F32, tag="qh")
                qhv = qh.rearrange("p (h d) -> p h d", h=NHL)
                tmp = sbA.tile([128, NHL, 64], F32, tag="tmp")
                nc.vector.tensor_tensor(
                    out=qhv[:, :, 0:64], in0=qv[:, :, 0:64], in1=cosB, op=ALU.mult)
                nc.vector.tensor_tensor(
                    out=tmp, in0=qv[:, :, 64:128], in1=sinB, op=ALU.mult)
                nc.vector.tensor_tensor(
                    out=qhv[:, :, 0:64], in0=qhv[:, :, 0:64], in1=tmp, op=ALU.add)
                nc.vector.tensor_tensor(
                    out=qhv[:, :, 64:128], in0=qv[:, :, 64:128], in1=cosB, op=ALU.mult)
                nc.vector.tensor_tensor(
                    out=tmp, in0=qv[:, :, 0:64], in1=sinB, op=ALU.mult)
                nc.vector.tensor_tensor(
                    out=qhv[:, :, 64:128], in0=qhv[:, :, 64:128], in1=tmp,
                    op=ALU.subtract)
                kv = kv_ps[:, 0:HD]
                kh = sbA.tile([128, HD], F32, tag="kh")
                ktmp = sbA.tile([128, 64], F32, tag="ktmp")
                cs1 = cos_sb[:, tb, :]
                sn1 = sin_sb[:, tb, :]
                nc.vector.tensor_tensor(
                    out=kh[:, 0:64], in0=kv[:, 0:64], in1=cs1, op=ALU.mult)
                nc.vector.tensor_tensor(
                    out=ktmp, in0=kv[:, 64:128], in1=sn1, op=ALU.mult)
                nc.vector.tensor_tensor(
                    out=kh[:, 0:64], in0=kh[:, 0:64], in1=ktmp, op=ALU.add)
                nc.vector.tensor_tensor(
                    out=kh[:, 64:128], in0=kv[:, 64:128], in1=cs1, op=ALU.mult)
                nc.vector.tensor_tensor(
                    out=ktmp, in0=kv[:, 0:64], in1=sn1, op=ALU.mult)
                nc.vector.tensor_tensor(
                    out=kh[:, 64:128], in0=kh[:, 64:128], in1=ktmp, op=ALU.subtract)

                # ---- RMSNorm: per-head sum of squares on the Scalar engine
                red = sbA.tile([128, NHL + 1], F32, tag="red")
                sqscr = sbA.tile([128, HD], F32, tag="sqscr")
                for hh in range(NHL):
                    nc.scalar.activation(
                        sqscr, qh[:, hh * 128:(hh + 1) * 128], AF.Square,
                        accum_out=red[:, hh:hh + 1])
                nc.scalar.activation(
                    sqscr, kh, AF.Square, accum_out=red[:, NHL:NHL + 1])

                # deferred finish of the previous block (all inputs one
                # block old -> no DVE FIFO stalls)
                if tb - 1 in chain:
                    finish_chain(tb - 1)
                chain[tb] = dict(tb=tb, red=red, qh=qh, kh=kh, e=e_sb,
                                 kv=kv_ps)

            finish_chain(TB - 1)

        # wproj loaded early so phase C does not stall on it
        wC = stk.enter_context(tc.tile_pool(name="wC", bufs=1))
        wp_sb = wC.tile([128, NHL, OC, 512], BF16)
        nc.gpsimd.dma_start(
            out=wp_sb,
            in_=wproj.rearrange("(h p) (oc o) -> p h oc o", p=128, o=512))

        # ---------------- phase B: attention ----------------
        # Globally software-pipelined: S/exp of group k+1 is emitted before
        # AV/denominator of group k, across iteration boundaries.
        # Denominator handling is deferred to once per head: yps/dps are
        # copied out of PSUM immediately (fast chunk turnover, PE never
        # waits on the reciprocal), then one DRAM round trip reshapes d to
        # partition-major for a cheap [128,16] reciprocal, and the scaled
        # yT write overlaps the next head's matmuls.
        with nc.named_scope("phaseB"), \
                tc.tile_pool(name="ptB", bufs=6) as ptB, \
                tc.tile_pool(name="ysB", bufs=2) as ysB, \
                tc.tile_pool(name="dsB", bufs=2) as dsB, \
                tc.tile_pool(name="bcB", bufs=8) as bcB, \
                tc.tile_pool(name="dpB", bufs=2) as dpB, \
                tc.tile_pool(name="psS", bufs=2, space="PSUM") as psS, \
                tc.tile_pool(name="psy", bufs=1, space="PSUM") as psy, \
                tc.tile_pool(name="psd", bufs=1, space="PSUM") as psd:

            def s_group(meta, g):
                hh, c, i0 = meta["hh"], meta["c"], meta["i0"]
                sps = psS.tile([128, 1024], F32, tag="sps", name="sps")
                pt = ptB.tile([128, 1024], BF16, tag="pt", name="pt")
                for s in range(2):
                    jb = 2 * g + s
                    io = max(0, 128 * jb - 512 * c)  # first causally-live col
                    nc.tensor.matmul(
                        sps[:, s * 512 + io:(s + 1) * 512],
                        lhsT=kT[:, jb * 128:(jb + 1) * 128],
                        rhs=qT[:, hh, i0 + io:i0 + 512],
                        start=True, stop=True)
                # one exp from the first live column: stale columns are never
                # read downstream (AV/dps start at io), so a single big exp
                # is cheaper than per-tile trimmed ones.
                io0 = max(0, 128 * (2 * g - 4 * c))
                nc.scalar.activation(
                    pt[:, io0:1024], sps[:, io0:1024], AF.Exp, scale=ISQ)
                for s in range(2):
                    jb = 2 * g + s
                    if jb >= 4 * c:  # diagonal block: zero j > i (tri mask)
                        io = 128 * (jb - 4 * c)
                        nc.vector.tensor_tensor(
                            out=pt[:, s * 512 + io:s * 512 + io + 128],
                            in0=pt[:, s * 512 + io:s * 512 + io + 128],
                            in1=triu_b, op=ALU.mult)
                meta["pts"][g] = pt

            def av_group(meta, g, is_last):
                pt = meta["pts"].pop(g)
                yps, dps = meta["yps"], meta["dps"]
                c = meta["c"]
                for s in range(2):
                    jb = 2 * g + s
                    io = max(0, 128 * jb - 512 * c)
                    if jb == meta["first_jb"]:
                        io = 0  # start matmul must cover the full chunk
                    nc.tensor.matmul(
                        yps[:, io:512], lhsT=vS[:, jb, :],
                        rhs=pt[:, s * 512 + io:(s + 1) * 512],
                        start=(jb == meta["first_jb"]),
                        stop=(jb == meta["last_jb"]))
                    nc.tensor.matmul(
                        dps[:, io:512], lhsT=ones_b,
                        rhs=pt[:, s * 512 + io:(s + 1) * 512],
                        start=(jb == meta["first_jb"]),
                        stop=(jb == meta["last_jb"]))
                if is_last:
                    hh, c = meta["hh"], meta["c"]
                    nc.vector.tensor_copy(out=meta["ysb"][:, c, :], in_=yps)
                    nc.vector.tensor_copy(
                        out=meta["dsb"][0:1, c, :], in_=dps[0:1, :])
                    if hh == NHL - 1:
                        # last head: staged per-chunk finalize so phase C is
                        # not serialized behind the whole head's round trip
                        s0 = c * 512
                        nc.sync.dma_start(
                            out=rscr[hh, s0:s0 + 512], in_=meta["dsb"][0:1, c, :])
                        dPc = dpB.tile([128, 4], F32, tag="dpc", name="dpc")
                        nc.sync.dma_start(
                            out=dPc,
                            in_=rscr[hh, s0:s0 + 512].rearrange(
                                "(p f) -> p f", p=128))
                        deferred.append(
                            [lambda hh=hh, c=c, dPc=dPc: fin_recip(hh, c, dPc),
                             lambda hh=hh, c=c, ysb=meta["ysb"]:
                                 fin_mult(hh, c, ysb)])
                    elif c == NCH - 1:
                        head_finalize(hh, meta["ysb"], meta["dsb"])

            def fin_recip(hh, c, dPc):
                s0 = c * 512
                rPc = dpB.tile([128, 4], F32, tag="rpc", name="rpc")
                nc.vector.reciprocal(rPc, dPc)
                nc.sync.dma_start(
                    out=rp2[hh, s0:s0 + 512].rearrange("(p f) -> p f", p=128),
                    in_=rPc)
                bca = bcB.tile([128, 512], F32, tag="bca", name="bca")
                nc.sync.dma_start(
                    out=bca, in_=_bcast(rp2[hh, s0:s0 + 512], 0, 128))
                bcas[(hh, c)] = bca

            def fin_mult(hh, c, ysb):
                nc.vector.tensor_tensor(
                    out=yT[:, hh, c * 512:(c + 1) * 512],
                    in0=ysb[:, c, :], in1=bcas.pop((hh, c)), op=ALU.mult)

            def head_finalize(hh, ysb, dsb):
                # d -> DRAM -> partition-major [128,16] -> recip -> DRAM ->
                # per-chunk broadcast; the yT scale multiplies are deferred
                # into the next head's chunk iterations so stalled DVE ops
                # never head-of-line-block the next head's PSUM copies.
                nc.sync.dma_start(out=rscr[hh, :], in_=dsb[0:1, :, :])
                dP = dpB.tile([128, T // 128], F32, tag="dp", name="dp")
                nc.sync.dma_start(
                    out=dP, in_=rscr[hh, :].rearrange("(p f) -> p f", p=128))
                rP = dpB.tile([128, T // 128], F32, tag="rp", name="rp")
                nc.vector.reciprocal(rP, dP)
                nc.sync.dma_start(
                    out=rp2[hh, :].rearrange("(p f) -> p f", p=128), in_=rP)
                for c2 in range(NCH):
                    bca = bcB.tile([128, 512], F32, tag="bca", name="bca")
                    nc.sync.dma_start(
                        out=bca,
                        in_=_bcast(rp2[hh, c2 * 512:(c2 + 1) * 512], 0, 128))
                    bcas[(hh, c2)] = bca
                    deferred.append(
                        [lambda hh=hh, c2=c2, ysb=ysb: fin_mult(hh, c2, ysb)])

            deferred = []  # lists of closures; one stage emitted per chunk
            bcas = {}

            def run_deferred():
                # emit a single deferred stage per chunk boundary: stalled
                # DVE ops must never queue up ahead of the copies that free
                # PSUM for the next chunk
                if deferred:
                    item = deferred[0]
                    item.pop(0)()
                    if not item:
                        deferred.pop(0)

            prev = None
            for hh in range(NHL):
                ysb = ysB.tile([128, NCH, 512], F32, tag="ysb", name="ysb")
                dsb = dsB.tile([1, NCH, 512], F32, tag="dsb", name="dsb")
                for c in range(NCH):
                    if hh == 0 and c == NCH - 1:
                        # blocks 14/15's q/k transposes, deferred from phase
                        # A: needed first by this c=3 chunk, and by now the
                        # phase-A DVE tail has long finished
                        for tbx in sorted(pend):
                            drain_pend(tbx)
                    run_deferred()
                    yps = psy.tile([128, 512], F32, tag="yps", name="yps")
                    dps = psd.tile([128, 512], F32, tag="dps", name="dps")
                    ngrp = (4 * c + 4) // 2
                    order = list(range(ngrp - 2, ngrp)) + list(range(ngrp - 2))
                    meta = dict(hh=hh, c=c, i0=c * 512, yps=yps, dps=dps,
                                ysb=ysb, dsb=dsb,
                                pts={}, first_jb=2 * order[0],
                                last_jb=2 * order[-1] + 1)
                    for idx, g in enumerate(order):
                        s_group(meta, g)
                        if prev is not None:
                            av_group(*prev)
                        prev = (meta, g, idx == len(order) - 1)
            av_group(*prev)
            while deferred:
                run_deferred()

        # ---------------- phase C: output projection ----------------
        with nc.named_scope("phaseC"), \
                tc.tile_pool(name="sbC", bufs=3) as sbC, \
                tc.tile_pool(name="psC", bufs=2, space="PSUM") as psC:
            for tb in range(TB):
                t0 = tb * 128
                for ocp in range(OC // 2):
                    o_ps = psC.tile([128, 1024], F32, tag="ops")
                    for oc2 in range(2):
                        oc = 2 * ocp + oc2
                        for hh in range(NHL):
                            nc.tensor.matmul(
                                o_ps[:, oc2 * 512:(oc2 + 1) * 512],
                                lhsT=yT[:, hh, t0:t0 + 128],
                                rhs=wp_sb[:, hh, oc, :],
                                start=(hh == 0), stop=(hh == NHL - 1))
                    o_sb = sbC.tile([128, 1024], BF16, tag="osb")
                    for oc2 in range(2):
                        oc = 2 * ocp + oc2
                        if oc2 == 0:
                            nc.scalar.copy(
                                out=o_sb[:, oc2 * 512:(oc2 + 1) * 512],
                                in_=o_ps[:, oc2 * 512:(oc2 + 1) * 512])
                        else:
                            nc.vector.tensor_copy(
                                out=o_sb[:, oc2 * 512:(oc2 + 1) * 512],
                                in_=o_ps[:, oc2 * 512:(oc2 + 1) * 512])
                        nc.sync.dma_start(
                            out=out[t0:t0 + 128, oc * 512:(oc + 1) * 512],
                            in_=o_sb[:, oc2 * 512:(oc2 + 1) * 512])

    nc.compile()
    return nc


_NC_CACHE = {}


def get_nc(T=2048):
    if T not in _NC_CACHE:
        _NC_CACHE[T] = build(T)
    return _NC_CACHE[T]


def make_in_maps(x, ve, cos, sin, Wq, Wk, Wv, Wproj, Wgate):
    """Shard full inputs into 8 per-core input maps (2 batch x 4 head groups)."""
    BF = ml_dtypes.bfloat16
    x = np.asarray(x, np.float32)
    ve = np.asarray(ve, np.float32)
    cosn = np.ascontiguousarray(np.asarray(cos, np.float32)[0, :, 0, :])
    sinn = np.ascontiguousarray(np.asarray(sin, np.float32)[0, :, 0, :])
    Wq = np.asarray(Wq, np.float32)
    Wk = np.asarray(Wk, np.float32)
    Wv = np.asarray(Wv, np.float32)
    Wproj = np.asarray(Wproj, np.float32)
    Wgate = np.asarray(Wgate, np.float32)
    in_maps = []
    for core in range(8):
        b, g = divmod(core, 4)
        wqkv = np.concatenate(
            [Wq[:, g * 512:(g + 1) * 512],
             Wk[:, g * 128:(g + 1) * 128],
             Wv[:, g * 128:(g + 1) * 128]], axis=1)
        in_maps.append({
            "xt": np.ascontiguousarray(x[b].T).astype(BF),
            "xg": np.ascontiguousarray(x[b][:, :GC]),
            "wqkv": np.ascontiguousarray(wqkv).astype(BF),
            "wproj": np.ascontiguousarray(Wproj[g * 512:(g + 1) * 512, :]).astype(BF),
            "wgate": np.ascontiguousarray(Wgate[:, g:g + 1]),
            "ve2": np.ascontiguousarray(2.0 * ve[b][:, g * 128:(g + 1) * 128]),
            "cosn": cosn,
            "sinn": sinn,
        })
    return in_maps


def run_cores(in_maps, trace=False, **kw):
    nc = get_nc(in_maps[0]["xg"].shape[0])
    return run_bass_kernel_spmd(nc, in_maps, core_ids=list(range(8)), trace=trace, **kw)


def kernel(**inputs):
    in_maps = make_in_maps(
        inputs["x"], inputs["ve"], inputs["cos"], inputs["sin"],
        inputs["Wq"], inputs["Wk"], inputs["Wv"], inputs["Wproj"], inputs["Wgate"])
    res = run_cores(in_maps)
    parts = [np.asarray(res.results[i]["out"], np.float32) for i in range(8)]
    out = np.stack([
        parts[0] + parts[1] + parts[2] + parts[3],
        parts[4] + parts[5] + parts[6] + parts[7],
    ]).astype(np.float32)
    return out


# revision 47
# speedup vs baseline: 1.0290x; 1.0083x over previous
"""Trainium2 Bass kernel for a causal self-attention block (GQA + gated value
embedding + RoPE + QK-RMSNorm), sharded over 8 NeuronCores.

Sharding: 8 cores = 2 (batch) x 4 (kv-head groups).  Each core computes, for
its batch b and head-group g (4 q-heads + 1 kv-head):
    q/k/v projections, gated ve addition, RoPE, RMSNorm, causal attention,
    and the partial output projection  y_g @ Wproj[g*512:(g+1)*512, :].
The host sums the 4 per-group partials for each batch (the Wproj
contraction distributes over head groups).

v2: bf16 operand datapath (FWL weight loads), host-side x transpose (no
on-device x transposes), 2-block-lagged q/k transposes (keeps the PE
streaming, HAM stays warm), DVE triangular-mask multiplies instead of
GpSimd affine_select, RMS square-sums and qhat scaling on the Scalar
engine, bf16 partial outputs.

Self-contained: hardcodes shapes; accepts FULL inputs, returns FULL output.
"""

from contextlib import ExitStack

import ml_dtypes
import numpy as np

import concourse.bacc as bacc
import concourse.bass as bass
import concourse.mybir as mybir
import concourse.tile as tile
from concourse.bass_utils import run_bass_kernel_spmd
from concourse.masks import make_identity

F32 = mybir.dt.float32
BF16 = mybir.dt.bfloat16
I32 = mybir.dt.int32
AF = mybir.ActivationFunctionType
ALU = mybir.AluOpType
AX = mybir.AxisListType

B, C, HD, NHL, GC = 2, 2048, 128, 4, 32  # NHL = local q heads per core
EPS = float(np.finfo(np.float32).eps)
ISQ = 1.0 / float(np.sqrt(128.0))
RSQRT_MAGIC = 0x5F3759DF


def _bcast(ap_, idx, count):
    """Insert a step-0 (broadcast) dim at position idx of the AP dims."""
    lst = [list(p) for p in ap_.ap]
    lst.insert(idx, [0, count])
    return bass.AP(ap_.tensor, ap_.offset, lst)


def build(T=2048):
    TB = T // 128   # token blocks
    CT = C // 128   # contraction tiles for qkv
    NCH = T // 512  # i-chunks for attention
    OC = C // 512   # output chunks for proj

    nc = bacc.Bacc("TRN2", target_bir_lowering=False, debug=False)
    xt = nc.dram_tensor("xt", [C, T], BF16, kind="ExternalInput")
    xg = nc.dram_tensor("xg", [T, GC], F32, kind="ExternalInput")
    wqkv = nc.dram_tensor("wqkv", [C, NHL * HD + 2 * HD], BF16, kind="ExternalInput")
    wproj = nc.dram_tensor("wproj", [NHL * HD, C], BF16, kind="ExternalInput")
    wgate = nc.dram_tensor("wgate", [GC, 1], F32, kind="ExternalInput")
    ve2 = nc.dram_tensor("ve2", [T, HD], F32, kind="ExternalInput")
    cosn = nc.dram_tensor("cosn", [T, 64], F32, kind="ExternalInput")
    sinn = nc.dram_tensor("sinn", [T, 64], F32, kind="ExternalInput")
    out = nc.dram_tensor("out", [T, C], BF16, kind="ExternalOutput")
    rscr = nc.dram_tensor("rscr", [NHL, T], F32)  # denominator bounce buffer
    rp2 = nc.dram_tensor("rp2", [NHL, T], F32)    # reciprocal bounce buffer

    QN = NHL * HD          # 512 q cols
    KVN = 2 * HD           # 256 k|v cols

    with ExitStack() as stk:
        tc = stk.enter_context(tile.TileContext(nc))
        gpool = stk.enter_context(tc.tile_pool(name="gconst", bufs=1))
        ident = gpool.tile([128, 128], F32)
        make_identity(nc, ident)
        identb = gpool.tile([128, 128], BF16)
        nc.vector.tensor_copy(out=identb, in_=ident)
        # full-width ones stationary: a [128,1] stationary (col_grp q0)
        # breaks LDWEIGHTS/matmul pipelining, costing ~190ns per jb tile
        ones_f = gpool.tile([128, 128], F32)
        nc.vector.memset(ones_f, 1.0)
        ones_b = gpool.tile([128, 128], BF16)
        nc.vector.tensor_copy(out=ones_b, in_=ones_f)
        # upper-triangular (keep j<=i) causal mask for diagonal 128x128 tiles
        triu_f = gpool.tile([128, 128], F32)
        nc.vector.memset(triu_f, 1.0)
        nc.gpsimd.affine_select(
            out=triu_f, in_=triu_f, pattern=[[1, 128]], compare_op=ALU.is_ge,
            fill=0.0, base=0, channel_multiplier=-1)
        triu_b = gpool.tile([128, 128], BF16)
        nc.vector.tensor_copy(out=triu_b, in_=triu_f)

        # PE warmup: dummy transposes so HAM reaches full clock while the
        # first DMAs land. Uses a memset scratch tile so the first transpose
        # depends only on one dependency-free DVE memset, not the identity
        # build chain.
        wz = gpool.tile([128, 128], BF16)
        nc.vector.memset(wz, 0.0)
        with tc.tile_pool(name="warm", bufs=2, space="PSUM") as warm:
            for _ in range(112):
                w_ps = warm.tile([128, 128], BF16, tag="wps", name="wps")
                nc.tensor.transpose(w_ps, wz, wz)

        persist = stk.enter_context(tc.tile_pool(name="persist", bufs=1))
        qT = persist.tile([128, NHL, T], BF16)   # [d, h, t]
        kT = persist.tile([128, T], BF16)        # [d, t]
        vS = persist.tile([128, TB, HD], BF16)   # [t%128, t//128, d]
        yT = persist.tile([128, NHL, T], BF16)   # [d, h, t]

        # qkh and pst stay open into phase B: the last two blocks' q/k
        # transposes are emitted inside phase B (they are only needed by the
        # c=3 chunks), so phase B's first S matmuls never wait on phase A's
        # DVE tail.
        qkh = stk.enter_context(tc.tile_pool(name="qkh", bufs=4))
        pst = stk.enter_context(tc.tile_pool(name="pst", bufs=2, space="PSUM"))

        pend = {}  # tb -> (qhat, khat) awaiting transpose into qT/kT

        def drain_pend(tb):
            pqh, pkh = pend.pop(tb)
            t0 = tb * 128
            for hh in range(NHL):
                tq_ps = pst.tile([128, 128], BF16, tag="tps")
                nc.tensor.transpose(
                    tq_ps, pqh[:, hh * 128:(hh + 1) * 128], identb)
                if hh % 2 == 0:
                    nc.scalar.copy(out=qT[:, hh, t0:t0 + 128], in_=tq_ps)
                else:
                    nc.vector.tensor_copy(out=qT[:, hh, t0:t0 + 128], in_=tq_ps)
            tk_ps = pst.tile([128, 128], BF16, tag="tps")
            nc.tensor.transpose(tk_ps, pkh, identb)
            nc.vector.tensor_copy(out=kT[:, t0:t0 + 128], in_=tk_ps)

        # ---------------- phase A: qkv + rope + rmsnorm + transposes --------
        with nc.named_scope("phaseA"), \
                tc.tile_pool(name="wA", bufs=1) as wA, \
                tc.tile_pool(name="xpA", bufs=3) as xpA, \
                tc.tile_pool(name="sbA", bufs=3) as sbA, \
                tc.tile_pool(name="psq", bufs=3, space="PSUM") as psq, \
                tc.tile_pool(name="pskv", bufs=3, space="PSUM") as pskv:
            # x pair 0 first (gates the first matmuls), then the small
            # tables the block-0 DVE chain needs, then the rest; qkv
            # weights stream per-ct on the gpsimd queue in parallel.
            xtr = xt.rearrange("(ct p) t -> p ct t", p=128)
            xpairs = {}

            def load_xpair(pr):
                xp = xpA.tile([128, CT, 256], BF16, tag="xp", name="xp")
                nc.sync.dma_start(out=xp, in_=xtr[:, :, pr * 256:pr * 256 + 256])
                xpairs[pr] = xp

            load_xpair(0)
            wgb_sb = wA.tile([128, GC], F32)
            nc.sync.dma_start(out=wgb_sb, in_=_bcast(wgate[:, 0], 0, 128))
            cos_sb = wA.tile([128, TB, 64], F32)
            nc.sync.dma_start(out=cos_sb, in_=cosn.rearrange("(tb p) d -> p tb d", p=128))
            sin_sb = wA.tile([128, TB, 64], F32)
            nc.sync.dma_start(out=sin_sb, in_=sinn.rearrange("(tb p) d -> p tb d", p=128))
            xg_sb = wA.tile([128, TB, GC], F32)
            nc.sync.dma_start(out=xg_sb, in_=xg.rearrange("(tb p) g -> p tb g", p=128))
            ve_sb = wA.tile([128, TB, HD], F32)
            nc.sync.dma_start(out=ve_sb, in_=ve2.rearrange("(tb p) d -> p tb d", p=128))
            load_xpair(1)
            wqkv_sb = wA.tile([128, CT, QN + KVN], BF16)
            wqkvr = wqkv.rearrange("(ct p) j -> p ct j", p=128)
            for ct in range(CT):
                nc.gpsimd.dma_start(out=wqkv_sb[:, ct, :], in_=wqkvr[:, ct, :])

            # Per-block chain is software-pipelined across blocks: the rope
            # and square-sum of block tb are emitted with tb's matmuls, but
            # the Newton rsqrt / qhat / khat / gate-finish / vS of block tb
            # are deferred one iteration so no engine FIFO ever waits on a
            # cross-engine result of the same block.
            chain = {}

            def finish_chain(tb):
                ch = chain.pop(tb)
                red, qh, kh = ch["red"], ch["qh"], ch["kh"]
                # m = mean + eps; rsqrt via bit-trick seed + 2 Newton steps
                nc.vector.tensor_scalar(
                    out=red, in0=red, scalar1=1.0 / 128.0, scalar2=EPS,
                    op0=ALU.mult, op1=ALU.add)
                rq = sbA.tile([128, NHL + 1], F32, tag="rq")
                rqi = rq.bitcast(I32)
                nc.vector.tensor_scalar(
                    out=rqi, in0=red.bitcast(I32), scalar1=1, scalar2=None,
                    op0=ALU.logical_shift_right)
                nc.vector.tensor_scalar(
                    out=rqi, in0=rqi, scalar1=-1, scalar2=RSQRT_MAGIC,
                    op0=ALU.mult, op1=ALU.add)
                nt = sbA.tile([128, NHL + 1], F32, tag="nt")
                for _ in range(2):
                    nc.vector.tensor_tensor(out=nt, in0=rq, in1=rq, op=ALU.mult)
                    nc.vector.tensor_tensor(out=nt, in0=nt, in1=red, op=ALU.mult)
                    nc.vector.tensor_scalar(
                        out=nt, in0=nt, scalar1=-0.5, scalar2=1.5,
                        op0=ALU.mult, op1=ALU.add)
                    nc.vector.tensor_tensor(out=rq, in0=rq, in1=nt, op=ALU.mult)
                qhat = qkh.tile([128, NHL * HD], BF16, tag="qhat")
                rqB = _bcast(rq[:, 0:NHL], 2, HD)
                nc.vector.tensor_tensor(
                    out=qhat.rearrange("p (h d) -> p h d", h=NHL),
                    in0=qh.rearrange("p (h d) -> p h d", h=NHL),
                    in1=rqB, op=ALU.mult)
                khat = qkh.tile([128, HD], BF16, tag="khat")
                nc.vector.tensor_scalar_mul(khat, kh, rq[:, NHL:NHL + 1])
                # gate finish + v = v_mm + sigmoid(z) * (2*ve)
                e_sb = ch["e"]
                nc.vector.tensor_scalar_add(e_sb, e_sb, 1.0)
                g_sb = sbA.tile([128, 1], F32, tag="gsb")
                nc.vector.reciprocal(g_sb, e_sb)
                nc.vector.scalar_tensor_tensor(
                    out=vS[:, ch["tb"], :], in0=ve_sb[:, ch["tb"], :],
                    scalar=g_sb, in1=ch["kv"][:, HD:2 * HD],
                    op0=ALU.mult, op1=ALU.add)
                pend[tb] = (qhat, khat)

            for tb in range(TB):
                pr, half = divmod(tb, 2)
                if pr + 1 not in xpairs and pr + 1 < TB // 2:
                    load_xpair(pr + 1)
                xp = xpairs[pr]
                q_ps = psq.tile([128, QN], F32, tag="qps")
                kv_ps = pskv.tile([128, KVN], F32, tag="kvps")
                for ct in range(CT):
                    xl = xp[:, ct, half * 128:half * 128 + 128]
                    nc.tensor.matmul(
                        q_ps, lhsT=xl, rhs=wqkv_sb[:, ct, 0:QN],
                        start=(ct == 0), stop=(ct == CT - 1))
                    nc.tensor.matmul(
                        kv_ps, lhsT=xl, rhs=wqkv_sb[:, ct, QN:QN + KVN],
                        start=(ct == 0), stop=(ct == CT - 1))
                # previous-previous block's qhat/khat -> qT/kT (PE transposes)
                if tb - 2 in pend:
                    drain_pend(tb - 2)
                if half == 1 and pr in xpairs:
                    del xpairs[pr]

                # gate part 1: z = x[:, :32] @ wgate (DVE), e = exp(-z) (Act)
                zg_sb = sbA.tile([128, 1], F32, tag="zg")
                zscr = sbA.tile([128, GC], F32, tag="zscr")
                nc.vector.scalar_tensor_tensor(
                    out=zscr, in0=xg_sb[:, tb, :], scalar=1.0, in1=wgb_sb,
                    op0=ALU.bypass, op1=ALU.mult, accum_out=zg_sb)
                e_sb = sbA.tile([128, 1], F32, tag="esb")
                nc.scalar.activation(e_sb, zg_sb, AF.Exp, scale=-1.0)

                # ---- RoPE on q (4 heads batched) and k ----
                cosB = _bcast(cos_sb[:, tb, :], 1, NHL)
                sinB = _bcast(sin_sb[:, tb, :], 1, NHL)
                qv = q_ps.rearrange("p (h d) -> p h d", h=NHL)
                qh = sbA.tile([128, NHL * HD], F32, tag="qh")
                qhv = qh.rearrange("p (h d) -> p h d", h=NHL)
                tmp = sbA.tile([128, NHL, 64], F32, tag="tmp")
                nc.vector.tensor_tensor(
                    out=qhv[:, :, 0:64], in0=qv[:, :, 0:64], in1=cosB, op=ALU.mult)
                nc.vector.tensor_tensor(
                    out=tmp, in0=qv[:, :, 64:128], in1=sinB, op=ALU.mult)
                nc.vector.tensor_tensor(
                    out=qhv[:, :, 0:64], in0=qhv[:, :, 0:64], in1=tmp, op=ALU.add)
                nc.vector.tensor_tensor(
                    out=qhv[:, :, 64:128], in0=qv[:, :, 64:128], in1=cosB, op=ALU.mult)
                nc.vector.tensor_tensor(
                    out=tmp, in0=qv[:, :, 0:64], in1=sinB, op=ALU.mult)
                nc.vector.tensor_tensor(
                    out=qhv[:, :, 64:128], in0=qhv[:, :, 64:128], in1=tmp,
                    op=ALU.subtract)
                kv = kv_ps[:, 0:HD]
                kh = sbA.tile([128, HD], F32, tag="kh")
                ktmp = sbA.tile([128, 64], F32, tag="ktmp")
                cs1 = cos_sb[:, tb, :]
                sn1 = sin_sb[:, tb, :]
                nc.vector.tensor_tensor(
                    out=kh[:, 0:64], in0=kv[:, 0:64], in1=cs1, op=ALU.mult)
                nc.vector.tensor_tensor(
                    out=ktmp, in0=kv[:, 64:128], in1=sn1, op=ALU.mult)
                nc.vector.tensor_tensor(
                    out=kh[:, 0:64], in0=kh[:, 0:64], in1=ktmp, op=ALU.add)
                nc.vector.tensor_tensor(
                    out=kh[:, 64:128], in0=kv[:, 64:128], in1=cs1, op=ALU.mult)
                nc.vector.tensor_tensor(
                    out=ktmp, in0=kv[:, 0:64], in1=sn1, op=ALU.mult)
                nc.vector.tensor_tensor(
                    out=kh[:, 64:128], in0=kh[:, 64:128], in1=ktmp, op=ALU.subtract)

                # ---- RMSNorm: per-head sum of squares on the Scalar engine
                red = sbA.tile([128, NHL + 1], F32, tag="red")
                sqscr = sbA.tile([128, HD], F32, tag="sqscr")
                for hh in range(NHL):
                    nc.scalar.activation(
                        sqscr, qh[:, hh * 128:(hh + 1) * 128], AF.Square,
                        accum_out=red[:, hh:hh + 1])
                nc.scalar.activation(
                    sqscr, kh, AF.Square, accum_out=red[:, NHL:NHL + 1])

                # deferred finish of the previous block (all inputs one
                # block old -> no DVE FIFO stalls)
                if tb - 1 in chain:
                    finish_chain(tb - 1)
                chain[tb] = dict(tb=tb, red=red, qh=qh, kh=kh, e=e_sb,
                                 kv=kv_ps)

            finish_chain(TB - 1)

        # wproj loaded early so phase C does not stall on it
        wC = stk.enter_context(tc.tile_pool(name="wC", bufs=1))
        wp_sb = wC.tile([128, NHL, OC, 512], BF16)
        nc.gpsimd.dma_start(
            out=wp_sb,
            in_=wproj.rearrange("(h p) (oc o) -> p h oc o", p=128, o=512))

        # ---------------- phase B: attention ----------------
        # Globally software-pipelined: S/exp of group k+1 is emitted before
        # AV/denominator of group k, across iteration boundaries.
        # Denominator handling is deferred to once per head: yps/dps are
        # copied out of PSUM immediately (fast chunk turnover, PE never
        # waits on the reciprocal), then one DRAM round trip reshapes d to
        # partition-major for a cheap [128,16] reciprocal, and the scaled
        # yT write overlaps the next head's matmuls.
        with nc.named_scope("phaseB"), \
                tc.tile_pool(name="ptB", bufs=6) as ptB, \
                tc.tile_pool(name="ysB", bufs=2) as ysB, \
                tc.tile_pool(name="dsB", bufs=2) as dsB, \
                tc.tile_pool(name="bcB", bufs=8) as bcB, \
                tc.tile_pool(name="dpB", bufs=2) as dpB, \
                tc.tile_pool(name="psS", bufs=2, space="PSUM") as psS, \
                tc.tile_pool(name="psy", bufs=1, space="PSUM") as psy, \
                tc.tile_pool(name="psd", bufs=1, space="PSUM") as psd:

            def s_group(meta, g):
                hh, c, i0 = meta["hh"], meta["c"], meta["i0"]
                sps = psS.tile([128, 1024], F32, tag="sps", name="sps")
                pt = ptB.tile([128, 1024], BF16, tag="pt", name="pt")
                for s in range(2):
                    jb = 2 * g + s
                    io = max(0, 128 * jb - 512 * c)  # first causally-live col
                    nc.tensor.matmul(
                        sps[:, s * 512 + io:(s + 1) * 512],
                        lhsT=kT[:, jb * 128:(jb + 1) * 128],
                        rhs=qT[:, hh, i0 + io:i0 + 512],
                        start=True, stop=True)
                # one exp from the first live column: stale columns are never
                # read downstream (AV/dps start at io), so a single big exp
                # is cheaper than per-tile trimmed ones.
                io0 = max(0, 128 * (2 * g - 4 * c))
                nc.scalar.activation(
                    pt[:, io0:1024], sps[:, io0:1024], AF.Exp, scale=ISQ)
                for s in range(2):
                    jb = 2 * g + s
                    if jb >= 4 * c:  # diagonal block: zero j > i (tri mask)
                        io = 128 * (jb - 4 * c)
                        nc.vector.tensor_tensor(
                            out=pt[:, s * 512 + io:s * 512 + io + 128],
                            in0=pt[:, s * 512 + io:s * 512 + io + 128],
                            in1=triu_b, op=ALU.mult)
                meta["pts"][g] = pt

            def av_group(meta, g, is_last):
                pt = meta["pts"].pop(g)
                yps, dps = meta["yps"], meta["dps"]
                c = meta["c"]
                for s in range(2):
                    jb = 2 * g + s
                    io = max(0, 128 * jb - 512 * c)
                    if jb == meta["first_jb"]:
                        io = 0  # start matmul must cover the full chunk
                    nc.tensor.matmul(
                        yps[:, io:512], lhsT=vS[:, jb, :],
                        rhs=pt[:, s * 512 + io:(s + 1) * 512],
                        start=(jb == meta["first_jb"]),
                        stop=(jb == meta["last_jb"]))
                    nc.tensor.matmul(
                        dps[:, io:512], lhsT=ones_b,
                        rhs=pt[:, s * 512 + io:(s + 1) * 512],
                        start=(jb == meta["first_jb"]),
                        stop=(jb == meta["last_jb"]))
                if is_last:
                    hh, c = meta["hh"], meta["c"]
                    nc.vector.tensor_copy(out=meta["ysb"][:, c, :], in_=yps)
                    nc.vector.tensor_copy(
                        out=meta["dsb"][0:1, c, :], in_=dps[0:1, :])
                    if hh == NHL - 1:
                        # last head: staged per-chunk finalize so phase C is
                        # not serialized behind the whole head's round trip
                        s0 = c * 512
                        nc.sync.dma_start(
                            out=rscr[hh, s0:s0 + 512], in_=meta["dsb"][0:1, c, :])
                        dPc = dpB.tile([128, 4], F32, tag="dpc", name="dpc")
                        nc.sync.dma_start(
                            out=dPc,
                            in_=rscr[hh, s0:s0 + 512].rearrange(
                                "(p f) -> p f", p=128))
                        deferred.append(
                            [lambda hh=hh, c=c, dPc=dPc: fin_recip(hh, c, dPc),
                             lambda hh=hh, c=c, ysb=meta["ysb"]:
                                 fin_mult(hh, c, ysb)])
                    elif c == NCH - 1:
                        head_finalize(hh, meta["ysb"], meta["dsb"])

            def fin_recip(hh, c, dPc):
                s0 = c * 512
                rPc = dpB.tile([128, 4], F32, tag="rpc", name="rpc")
                nc.vector.reciprocal(rPc, dPc)
                nc.sync.dma_start(
                    out=rp2[hh, s0:s0 + 512].rearrange("(p f) -> p f", p=128),
                    in_=rPc)
                bca = bcB.tile([128, 512], F32, tag="bca", name="bca")
                nc.sync.dma_start(
                    out=bca, in_=_bcast(rp2[hh, s0:s0 + 512], 0, 128))
                bcas[(hh, c)] = bca

            def fin_mult(hh, c, ysb):
                nc.vector.tensor_tensor(
                    out=yT[:, hh, c * 512:(c + 1) * 512],
                    in0=ysb[:, c, :], in1=bcas.pop((hh, c)), op=ALU.mult)

            def head_finalize(hh, ysb, dsb):
                # d -> DRAM -> partition-major [128,16] -> recip -> DRAM ->
                # per-chunk broadcast; the yT scale multiplies are deferred
                # into the next head's chunk iterations so stalled DVE ops
                # never head-of-line-block the next head's PSUM copies.
                nc.sync.dma_start(out=rscr[hh, :], in_=dsb[0:1, :, :])
                dP = dpB.tile([128, T // 128], F32, tag="dp", name="dp")
                nc.sync.dma_start(
                    out=dP, in_=rscr[hh, :].rearrange("(p f) -> p f", p=128))
                rP = dpB.tile([128, T // 128], F32, tag="rp", name="rp")
                nc.vector.reciprocal(rP, dP)
                nc.sync.dma_start(
                    out=rp2[hh, :].rearrange("(p f) -> p f", p=128), in_=rP)
                for c2 in range(NCH):
                    bca = bcB.tile([128, 512], F32, tag="bca", name="bca")
                    nc.sync.dma_start(
                        out=bca,
                        in_=_bcast(rp2[hh, c2 * 512:(c2 + 1) * 512], 0, 128))
                    bcas[(hh, c2)] = bca
                    deferred.append(
                        [lambda hh=hh, c2=c2, ysb=ysb: fin_mult(hh, c2, ysb)])

            deferred = []  # lists of closures; one stage emitted per chunk
            bcas = {}

            def run_deferred():
                # emit a single deferred stage per chunk boundary: stalled
                # DVE ops must never queue up ahead of the copies that free
                # PSUM for the next chunk
                if deferred:
                    item = deferred[0]
                    item.pop(0)()
                    if not item:
                        deferred.pop(0)

            pending = []  # 2-deep software pipeline: S/exp runs two groups
            # ahead of AV so the Act exp latency is always fully covered
            for hh in range(NHL):
                ysb = ysB.tile([128, NCH, 512], F32, tag="ysb", name="ysb")
                dsb = dsB.tile([1, NCH, 512], F32, tag="dsb", name="dsb")
                for c in range(NCH):
                    if hh == 0 and c == NCH - 1:
                        # blocks 14/15's q/k transposes, deferred from phase
                        # A: needed first by this c=3 chunk, and by now the
                        # phase-A DVE tail has long finished
                        for tbx in sorted(pend):
                            drain_pend(tbx)
                    yps = psy.tile([128, 512], F32, tag="yps", name="yps")
                    dps = psd.tile([128, 512], F32, tag="dps", name="dps")
                    ngrp = (4 * c + 4) // 2
                    order = list(range(ngrp - 2, ngrp)) + list(range(ngrp - 2))
                    meta = dict(hh=hh, c=c, i0=c * 512, yps=yps, dps=dps,
                                ysb=ysb, dsb=dsb,
                                pts={}, first_jb=2 * order[0],
                                last_jb=2 * order[-1] + 1)
                    for idx, g in enumerate(order):
                        s_group(meta, g)
                        if len(pending) >= 2:
                            av_group(*pending.pop(0))
                        run_deferred()
                        pending.append((meta, g, idx == len(order) - 1))
            while pending:
                av_group(*pending.pop(0))
            while deferred:
                run_deferred()

        # ---------------- phase C: output projection ----------------
        with nc.named_scope("phaseC"), \
                tc.tile_pool(name="sbC", bufs=3) as sbC, \
                tc.tile_pool(name="psC", bufs=2, space="PSUM") as psC:
            for tb in range(TB):
                t0 = tb * 128
                for ocp in range(OC // 2):
                    o_ps = psC.tile([128, 1024], F32, tag="ops")
                    for oc2 in range(2):
                        oc = 2 * ocp + oc2
                        for hh in range(NHL):
                            nc.tensor.matmul(
                                o_ps[:, oc2 * 512:(oc2 + 1) * 512],
                                lhsT=yT[:, hh, t0:t0 + 128],
                                rhs=wp_sb[:, hh, oc, :],
                                start=(hh == 0), stop=(hh == NHL - 1))
                    o_sb = sbC.tile([128, 1024], BF16, tag="osb")
                    for oc2 in range(2):
                        oc = 2 * ocp + oc2
                        if oc2 == 0:
                            nc.scalar.copy(
                                out=o_sb[:, oc2 * 512:(oc2 + 1) * 512],
                                in_=o_ps[:, oc2 * 512:(oc2 + 1) * 512])
                        else:
                            nc.vector.tensor_copy(
                                out=o_sb[:, oc2 * 512:(oc2 + 1) * 512],
                                in_=o_ps[:, oc2 * 512:(oc2 + 1) * 512])
                        nc.sync.dma_start(
                            out=out[t0:t0 + 128, oc * 512:(oc + 1) * 512],
                            in_=o_sb[:, oc2 * 512:(oc2 + 1) * 512])

    nc.compile()
    return nc


_NC_CACHE = {}


def get_nc(T=2048):
    if T not in _NC_CACHE:
        _NC_CACHE[T] = build(T)
    return _NC_CACHE[T]


def make_in_maps(x, ve, cos, sin, Wq, Wk, Wv, Wproj, Wgate):
    """Shard full inputs into 8 per-core input maps (2 batch x 4 head groups)."""
    BF = ml_dtypes.bfloat16
    x = np.asarray(x, np.float32)
    ve = np.asarray(ve, np.float32)
    cosn = np.ascontiguousarray(np.asarray(cos, np.float32)[0, :, 0, :])
    sinn = np.ascontiguousarray(np.asarray(sin, np.float32)[0, :, 0, :])
    Wq = np.asarray(Wq, np.float32)
    Wk = np.asarray(Wk, np.float32)
    Wv = np.asarray(Wv, np.float32)
    Wproj = np.asarray(Wproj, np.float32)
    Wgate = np.asarray(Wgate, np.float32)
    in_maps = []
    for core in range(8):
        b, g = divmod(core, 4)
        wqkv = np.concatenate(
            [Wq[:, g * 512:(g + 1) * 512],
             Wk[:, g * 128:(g + 1) * 128],
             Wv[:, g * 128:(g + 1) * 128]], axis=1)
        in_maps.append({
            "xt": np.ascontiguousarray(x[b].T).astype(BF),
            "xg": np.ascontiguousarray(x[b][:, :GC]),
            "wqkv": np.ascontiguousarray(wqkv).astype(BF),
            "wproj": np.ascontiguousarray(Wproj[g * 512:(g + 1) * 512, :]).astype(BF),
            "wgate": np.ascontiguousarray(Wgate[:, g:g + 1]),
            "ve2": np.ascontiguousarray(2.0 * ve[b][:, g * 128:(g + 1) * 128]),
            "cosn": cosn,
            "sinn": sinn,
        })
    return in_maps


def run_cores(in_maps, trace=False, **kw):
    nc = get_nc(in_maps[0]["xg"].shape[0])
    return run_bass_kernel_spmd(nc, in_maps, core_ids=list(range(8)), trace=trace, **kw)


def kernel(**inputs):
    in_maps = make_in_maps(
        inputs["x"], inputs["ve"], inputs["cos"], inputs["sin"],
        inputs["Wq"], inputs["Wk"], inputs["Wv"], inputs["Wproj"], inputs["Wgate"])
    res = run_cores(in_maps)
    parts = [np.asarray(res.results[i]["out"], np.float32) for i in range(8)]
    out = np.stack([
        parts[0] + parts[1] + parts[2] + parts[3],
        parts[4] + parts[5] + parts[6] + parts[7],
    ]).astype(np.float32)
    return out
